# revision 1
# baseline (speedup 1.0000x reference)
"""Expert-parallel MoE kernel for Trainium2 (8 NeuronCores).

Strategy (matches the expert-parallel sharding hint):
  - Router is evaluated on host with the exact same jax ops as the
    reference (same backend) so top-k decisions match bit-for-bit.
  - Tokens are dispatched (gathered) per expert on host; each of the 8
    cores owns one expert's weights and runs a fused MLP
        Y = (silu(X @ G^T) * (X @ U^T)) @ D^T
    over its gathered tokens in bf16 (fp32 PSUM accumulate).
  - Outputs are combined on host: out[token] += mean_w[e] * Y_e[row].

v2 kernel-side changes vs v1 (same math, same numerics):
  - gate+up packed into one DRAM tensor -> 1 DMA per I-block (was 2).
  - X^T packed per token-tile -> 1 fat DMA per tile (was 8).
  - D^T loaded with a single DMA (was 32), issued after the first
    tile's first weight blocks so PE compute starts immediately.
  - Next tile's X + first weight blocks prefetched before stage 2 of
    the current tile, removing the inter-tile PE bubble.
  - y written back via one DMA per 128-token chunk (was 2).
"""

import sys
from contextlib import ExitStack

if "/opt/trn_rl_repo" not in sys.path:
    sys.path.insert(0, "/opt/trn_rl_repo")

import ml_dtypes
import numpy as np

import concourse.bacc as bacc
import concourse.mybir as mybir
import concourse.tile as tile
from concourse.bass_utils import run_bass_kernel_spmd

B, S, H, I, E, TOPK = 4, 2048, 1024, 4096, 8, 2
T = B * S
KCH = H // 128   # 8 contraction chunks over H
IB = I // 128    # 32 blocks over I
CT = 512         # token-tile capacity (PSUM free-dim limit)
BF16 = mybir.dt.bfloat16
F32 = mybir.dt.float32

_prog_cache: dict[tuple, object] = {}


def _ctiles(C):
    """Split capacity C into free-dim tiles (multiples of 128, <= 512).

    Short tiles give the PE accumulation chains too little slack to hide
    cross-engine latency, so avoid tails below 384: e.g. 2176 ->
    [512, 512, 384, 384, 384] rather than [512, 512, 512, 512, 128]."""
    r = C % CT
    sizes = [CT] * (C // CT)
    if r == 128 and len(sizes) >= 2:
        sizes = sizes[:-2] + [384, 384, 384]
    elif r == 256 and len(sizes) >= 1:
        sizes = sizes[:-1] + [384, 384]
    elif r:
        sizes.append(r)
    out, c = [], 0
    for s in sizes:
        out.append((c, s))
        c += s
    return out


def build_program(C, reps=1):
    key = (C, reps)
    if key in _prog_cache:
        return _prog_cache[key]
    nc = bacc.Bacc("TRN2", target_bir_lowering=False, debug=False, num_devices=8)

    NT = len(_ctiles(C))
    xt_d = nc.dram_tensor("xt", [NT, 128, KCH, CT], BF16, kind="ExternalInput").ap()
    gu_d = nc.dram_tensor("gu", [IB, 128, 2, KCH, 128], BF16, kind="ExternalInput").ap()
    dt_d = nc.dram_tensor("dt", [128, IB, H], BF16, kind="ExternalInput").ap()
    y_d = nc.dram_tensor("y", [C, H], F32, kind="ExternalOutput").ap()

    with tile.TileContext(nc) as tc:
        with ExitStack() as stack:
            if reps > 1:
                stack.enter_context(tc.For_i(0, reps, 1))
            _emit_body(nc, tc, stack, C, xt_d, gu_d, dt_d, y_d)

    nc.compile()
    _prog_cache[key] = nc
    return nc


def _dt_chunks_at(ib):
    """Schedule of D^T chunk loads across tile-0 stage-1 I-blocks: singles
    on ib 6..25, doubles on 26..31 (after gate/up prefetch has wound down)."""
    if 6 <= ib <= 25:
        return [ib - 6]
    if 26 <= ib <= 31:
        j = 20 + 2 * (ib - 26)
        return [j, j + 1]
    return []


def _emit_body(nc, tc, stack, C, xt_d, gu_d, dt_d, y_d):
    tiles = _ctiles(C)
    NT = len(tiles)
    silu = mybir.ActivationFunctionType.Silu

    with (
        tc.tile_pool(name="wpool", bufs=8) as wpool,
        tc.tile_pool(name="xpool", bufs=2) as xpool,
        tc.tile_pool(name="dpool", bufs=1) as dpool,
        tc.tile_pool(name="hpool", bufs=2) as hpool,
        tc.tile_pool(name="spool", bufs=3) as spool,
        tc.tile_pool(name="ypool", bufs=3) as ypool,
        tc.tile_pool(name="psum", bufs=2, space="PSUM") as psum,
    ):
        gus = {}

        def gu_load(t, ib):
            g = wpool.tile([128, 2, KCH, 128], BF16, tag="gu")
            nc.sync.dma_start(g[:], gu_d[ib])
            gus[(t, ib)] = g

        PF = 6  # gate/up prefetch depth (blocks in flight ahead of the PE)
        xts = [None] * NT
        xts[0] = xpool.tile([128, KCH, CT], BF16, tag="xt", name="xt0")
        nc.scalar.dma_start(xts[0][:], xt_d[0])
        for j in range(PF):
            gu_load(0, j)

        # D^T resident for the whole rep: [128, IB, H] bf16, loaded in
        # 32 chunks interleaved through tile 0's stage 1 so the transfers
        # never starve the gate/up weight stream.
        dtt = dpool.tile([128, IB, H], BF16, tag="dt")

        for t, (c0, cs) in enumerate(tiles):
            xt = xts[t]
            hhs = []
            for ib in range(IB):
                gu = gus.pop((t, ib))
                if ib + PF < IB:
                    gu_load(t, ib + PF)
                a1 = psum.tile([128, CT], F32, tag="a1", bufs=3)
                for k in range(KCH):
                    nc.tensor.matmul(
                        a1[:, :cs], gu[:, 0, k, :], xt[:, k, :cs],
                        start=(k == 0), stop=(k == KCH - 1),
                    )
                a2 = psum.tile([128, CT], F32, tag="a2", bufs=3)
                for k in range(KCH):
                    nc.tensor.matmul(
                        a2[:, :cs], gu[:, 1, k, :], xt[:, k, :cs],
                        start=(k == 0), stop=(k == KCH - 1),
                    )
                sl = spool.tile([128, CT], F32, tag="silu")
                nc.scalar.activation(sl[:, :cs], a1[:, :cs], silu)
                if ib == 20 and t + 1 < NT:
                    # prefetch next tile's tokens mid-stage-1; the
                    # Activation queue has transfer slack here and the
                    # coarse tile-level dep gates stage 1 of t+1 on this
                    xts[t + 1] = xpool.tile([128, KCH, CT], BF16, tag="xt", name="xtn")
                    nc.scalar.dma_start(xts[t + 1][:], xt_d[t + 1])
                if t == 0 and ib >= 6:
                    # D^T chunks ride the Activation HWDGE queue (the SP
                    # queue is saturated by the gate/up stream; SWDGE via
                    # gpsimd measured far slower than its cost model).
                    # Start at ib>=6 so the cross-rep WAR wait (previous
                    # rep's stage-2 reads of dtt) is already satisfied and
                    # never blocks the silu stream queued behind it.
                    for j in _dt_chunks_at(ib):
                        nc.scalar.dma_start(dtt[:, j, :], dt_d[:, j, :])
                hh = hpool.tile([128, CT], BF16, tag=f"hh{ib}")
                nc.vector.tensor_mul(hh[:, :cs], sl[:, :cs], a2[:, :cs])
                hhs.append(hh)

            # prefetch next tile's inputs before stage 2 keeps PE fed
            # across the tile boundary
            if t + 1 < NT:
                for j in range(PF):
                    gu_load(t + 1, j)

            # stage 2: Y[c, h] = Hh @ D^T  (contract I)
            for cs0 in range(0, cs, 128):
                yt = ypool.tile([128, H], F32, tag="yt")
                for hi, h0 in enumerate(range(0, H, 512)):
                    py = psum.tile([128, 512], F32, tag="py")
                    for ic in range(IB):
                        nc.tensor.matmul(
                            py[:],
                            hhs[ic][:, cs0 : cs0 + 128],
                            dtt[:, ic, h0 : h0 + 512],
                            start=(ic == 0), stop=(ic == IB - 1),
                        )
                    nc.scalar.copy(yt[:, h0 : h0 + 512], py[:])
                    nc.scalar.dma_start(
                        y_d[c0 + cs0 : c0 + cs0 + 128, h0 : h0 + 512],
                        yt[:, h0 : h0 + 512],
                    )


def _routing(x, router_w):
    """Replicate the reference's routing decisions with identical jax ops."""
    import jax
    import jax.numpy as jnp

    xf = jnp.asarray(x).reshape(-1, H)
    logits = xf @ jnp.asarray(router_w).T
    probs = jax.nn.softmax(logits, axis=-1)
    topk_p, topk_i = jax.lax.top_k(probs, TOPK)
    topk_p = topk_p / topk_p.sum(axis=-1, keepdims=True)
    return np.asarray(topk_p), np.asarray(topk_i)


def prepare(x, router_w, gate_w, up_w, down_w):
    """Host-side dispatch: returns (nc, in_maps, combine) where combine maps
    the per-core device outputs to the full [B,S,H] result."""
    topk_p, topk_i = _routing(x, router_w)
    xf = np.ascontiguousarray(np.asarray(x, dtype=np.float32).reshape(T, H))

    idxs, weights = [], []
    for e in range(E):
        sel = topk_i == e
        mask = sel.any(axis=-1)
        w_tok = (topk_p * sel).sum(axis=-1)
        cnt = int(mask.sum())
        mean_w = float(w_tok.sum() / max(cnt, 1)) if cnt > 0 else 0.0
        idxs.append(np.nonzero(mask)[0])
        weights.append(np.float32(mean_w))

    cmax = max(len(ix) for ix in idxs)
    C = ((cmax + 127) // 128) * 128
    NT = len(_ctiles(C))

    xf_bf = xf.astype(ml_dtypes.bfloat16)
    in_maps = []
    for e in range(E):
        ix = idxs[e]
        # X^T per tile: [NT, 128(h-sub), KCH(k), CT(c)], h = 128k + p.
        # Tile t covers tokens [c0, c0+CT) of the padded stream (tiles can
        # be shorter than CT; the kernel only reads [:cs] of each).
        xpad = np.zeros((C + CT, H), dtype=ml_dtypes.bfloat16)
        xpad[: len(ix)] = xf_bf[ix]
        xt = np.ascontiguousarray(
            np.stack(
                [
                    xpad[c0 : c0 + CT].reshape(CT, KCH, 128).transpose(2, 1, 0)
                    for c0, cs in _ctiles(C)
                ]
            )
        )
        # G^T/U^T packed together: [IB, 128(p), 2, KCH(k), 128(i)], h = 128k+p
        gT = np.asarray(gate_w[e], dtype=np.float32).T.astype(ml_dtypes.bfloat16)
        uT = np.asarray(up_w[e], dtype=np.float32).T.astype(ml_dtypes.bfloat16)
        gt = gT.reshape(KCH, 128, IB, 128).transpose(2, 1, 0, 3)
        ut = uT.reshape(KCH, 128, IB, 128).transpose(2, 1, 0, 3)
        gu = np.ascontiguousarray(np.stack([gt, ut], axis=2))
        # D^T: [128(p over I-sub), IB, H] with i = 128*ic + p
        dT = np.asarray(down_w[e], dtype=np.float32).T.astype(ml_dtypes.bfloat16)
        dt = np.ascontiguousarray(dT.reshape(IB, 128, H).transpose(1, 0, 2))
        in_maps.append({"xt": xt, "gu": gu, "dt": dt})

    nc = build_program(C)

    def combine(results):
        out = np.zeros((T, H), dtype=np.float32)
        for e in range(E):
            ix = idxs[e]
            y = results[e]["y"]
            out[ix] += weights[e] * y[: len(ix)]
        return out.reshape(B, S, H)

    return nc, in_maps, combine


def kernel(x, router_w, gate_w, up_w, down_w):
    nc, in_maps, combine = prepare(x, router_w, gate_w, up_w, down_w)
    res = run_bass_kernel_spmd(nc, in_maps, list(range(8)))
    return combine(res.results)



# revision 5
# speedup vs baseline: 1.0123x; 1.0123x over previous
"""Expert-parallel MoE kernel for Trainium2 (8 NeuronCores).

Strategy (matches the expert-parallel sharding hint):
  - Router is evaluated on host with the exact same jax ops as the
    reference (same backend) so top-k decisions match bit-for-bit.
  - Tokens are dispatched (gathered) per expert on host; each of the 8
    cores owns one expert's weights and runs a fused MLP
        Y = (silu(X @ G^T) * (X @ U^T)) @ D^T
    over its gathered tokens in bf16 (fp32 PSUM accumulate).
  - Outputs are combined on host: out[token] += mean_w[e] * Y_e[row].

v2 kernel-side changes vs v1 (same math, same numerics):
  - gate+up packed into one DRAM tensor -> 1 DMA per I-block (was 2).
  - X^T packed per token-tile -> 1 fat DMA per tile (was 8).
  - D^T loaded with a single DMA (was 32), issued after the first
    tile's first weight blocks so PE compute starts immediately.
  - Next tile's X + first weight blocks prefetched before stage 2 of
    the current tile, removing the inter-tile PE bubble.
  - y written back via one DMA per 128-token chunk (was 2).
"""

import sys
from contextlib import ExitStack

if "/opt/trn_rl_repo" not in sys.path:
    sys.path.insert(0, "/opt/trn_rl_repo")

import ml_dtypes
import numpy as np

import concourse.bacc as bacc
import concourse.mybir as mybir
import concourse.tile as tile
from concourse.bass_utils import run_bass_kernel_spmd

B, S, H, I, E, TOPK = 4, 2048, 1024, 4096, 8, 2
T = B * S
KCH = H // 128   # 8 contraction chunks over H
IB = I // 128    # 32 blocks over I
CT = 512         # token-tile capacity (PSUM free-dim limit)
BF16 = mybir.dt.bfloat16
F32 = mybir.dt.float32

_prog_cache: dict[tuple, object] = {}


def _ctiles(C):
    """Split capacity C into free-dim tiles (multiples of 128, <= 512).

    Short tiles give the PE accumulation chains too little slack to hide
    cross-engine latency, so avoid tails below 384: e.g. 2176 ->
    [512, 512, 384, 384, 384] rather than [512, 512, 512, 512, 128]."""
    r = C % CT
    sizes = [CT] * (C // CT)
    if r == 128 and len(sizes) >= 2:
        sizes = sizes[:-2] + [384, 384, 384]
    elif r == 256 and len(sizes) >= 1:
        sizes = sizes[:-1] + [384, 384]
    elif r:
        sizes.append(r)
    out, c = [], 0
    for s in sizes:
        out.append((c, s))
        c += s
    return out


V3 = True  # pairs restructure: stream gate/up weights once per tile-PAIR


def build_program(C, reps=1, unroll=False):
    key = (C, reps, unroll, V3)
    if key in _prog_cache:
        return _prog_cache[key]
    nc = bacc.Bacc("TRN2", target_bir_lowering=False, debug=False, num_devices=8)

    NT = len(_ctiles(C))
    xt_d = nc.dram_tensor("xt", [NT, 128, KCH, CT], BF16, kind="ExternalInput").ap()
    gu_d = nc.dram_tensor("gu", [IB, 128, 2, KCH, 128], BF16, kind="ExternalInput").ap()
    dt_d = nc.dram_tensor("dt", [128, IB, H], BF16, kind="ExternalInput").ap()
    y_d = nc.dram_tensor("y", [C, H], F32, kind="ExternalOutput").ap()

    emit = _emit_body_v3 if V3 else _emit_body
    with tile.TileContext(nc) as tc:
        with ExitStack() as stack:
            if reps > 1 and not unroll:
                stack.enter_context(tc.For_i(0, reps, 1))
            for _ in range(reps if unroll else 1):
                emit(nc, tc, stack, C, xt_d, gu_d, dt_d, y_d)

    nc.compile()
    _prog_cache[key] = nc
    return nc


def _groups(C):
    """Group _ctiles into pairs (plus a possible trailing single): each
    group shares one pass of the gate/up weight stream."""
    ts = _ctiles(C)
    out, i = [], 0
    while i < len(ts):
        out.append(tuple(ts[i : i + 2]))
        i += 2
    return out


def _emit_body_v3(nc, tc, stack, C, xt_d, gu_d, dt_d, y_d):
    """Pairs version: stage-1 processes two 512-token tiles per pass of the
    gate/up stream; each stationary weight block feeds back-to-back matmuls
    for both tiles. PSUM: a1A,a1B,a2A,a2B x bufs=2 = 8 banks; stage-2 py
    allocs share tags a1A/a1B (rotation provides the WAR ordering)."""
    groups = _groups(C)
    tiles = _ctiles(C)
    NT = len(tiles)
    silu = mybir.ActivationFunctionType.Silu
    t_of = {}  # tile index -> (group idx, pos in group)
    ti = 0
    for gi, g in enumerate(groups):
        for pos in range(len(g)):
            t_of[ti] = (gi, pos)
            ti += 1

    with (
        tc.tile_pool(name="wpool", bufs=6) as wpool,
        tc.tile_pool(name="xpool", bufs=4) as xpool,
        tc.tile_pool(name="dpool", bufs=1) as dpool,
        tc.tile_pool(name="hpool", bufs=1) as hpool,
        tc.tile_pool(name="spool", bufs=3) as spool,
        tc.tile_pool(name="ypool", bufs=3) as ypool,
        tc.tile_pool(name="psum", bufs=2, space="PSUM") as psum,
    ):
        gus = {}

        def gu_load(gi, ib):
            g = wpool.tile([128, 2, KCH, 128], BF16, tag="gu")
            nc.sync.dma_start(g[:], gu_d[ib])
            gus[(gi, ib)] = g

        PF = 6
        xts = [None] * NT
        xts[0] = xpool.tile([128, KCH, CT], BF16, tag="xt", name="xt0")
        nc.scalar.dma_start(xts[0][:], xt_d[0])
        if NT > 1:
            xts[1] = xpool.tile([128, KCH, CT], BF16, tag="xt", name="xt1")
            nc.scalar.dma_start(xts[1][:], xt_d[1])
        for j in range(PF):
            gu_load(0, j)

        dtt = dpool.tile([128, IB, H], BF16, tag="dt")

        t0 = 0  # first tile index of the current group
        for gi, grp in enumerate(groups):
            npos = len(grp)
            hhs = [[] for _ in range(npos)]
            for ib in range(IB):
                gu = gus.pop((gi, ib))
                if ib + PF < IB:
                    gu_load(gi, ib + PF)
                a1s, a2s = [], []
                for p in range(npos):
                    a1s.append(psum.tile([128, CT], F32, tag=f"a1{p}", bufs=2, name=f"a1{p}"))
                    a2s.append(psum.tile([128, CT], F32, tag=f"a2{p}", bufs=2, name=f"a2{p}"))
                # one pass of the gu stream serves every tile in the group
                for p in range(npos):
                    cs = grp[p][1]
                    for k in range(KCH):
                        nc.tensor.matmul(
                            a1s[p][:, :cs], gu[:, 0, k, :], xts[t0 + p][:, k, :cs],
                            start=(k == 0), stop=(k == KCH - 1),
                        )
                for p in range(npos):
                    cs = grp[p][1]
                    for k in range(KCH):
                        nc.tensor.matmul(
                            a2s[p][:, :cs], gu[:, 1, k, :], xts[t0 + p][:, k, :cs],
                            start=(k == 0), stop=(k == KCH - 1),
                        )
                for p in range(npos):
                    cs = grp[p][1]
                    sl = spool.tile([128, CT], F32, tag="silu")
                    nc.scalar.activation(sl[:, :cs], a1s[p][:, :cs], silu)
                    if gi == 0 and p == npos - 1 and ib >= 6:
                        for j in _dt_chunks_at(ib):
                            nc.scalar.dma_start(dtt[:, j, :], dt_d[:, j, :])
                    hh = hpool.tile([128, CT], BF16, tag=f"hh{ib}p{p}")
                    nc.vector.tensor_mul(hh[:, :cs], sl[:, :cs], a2s[p][:, :cs])
                    hhs[p].append(hh)
                if ib == 20:
                    # prefetch next group's x tiles mid-stage-1
                    for nt in range(t0 + npos, min(t0 + npos + 2, NT)):
                        xts[nt] = xpool.tile([128, KCH, CT], BF16, tag="xt", name=f"xt{nt}")
                        nc.scalar.dma_start(xts[nt][:], xt_d[nt])

            if gi + 1 < len(groups):
                for j in range(PF):
                    gu_load(gi + 1, j)

            # stage 2 per tile of the group
            for p in range(npos):
                c0, cs = grp[p]
                for cs0 in range(0, cs, 128):
                    yt = ypool.tile([128, H], F32, tag="yt")
                    for hi, h0 in enumerate(range(0, H, 512)):
                        py = psum.tile([128, 512], F32, tag=f"a1{hi % npos}", bufs=2)
                        for ic in range(IB):
                            nc.tensor.matmul(
                                py[:],
                                hhs[p][ic][:, cs0 : cs0 + 128],
                                dtt[:, ic, h0 : h0 + 512],
                                start=(ic == 0), stop=(ic == IB - 1),
                            )
                        nc.scalar.copy(yt[:, h0 : h0 + 512], py[:])
                        nc.scalar.dma_start(
                            y_d[c0 + cs0 : c0 + cs0 + 128, h0 : h0 + 512],
                            yt[:, h0 : h0 + 512],
                        )
            t0 += npos


def _dt_chunks_at(ib):
    """Schedule of D^T chunk loads across tile-0 stage-1 I-blocks: singles
    on ib 6..25, doubles on 26..31 (after gate/up prefetch has wound down)."""
    if 6 <= ib <= 25:
        return [ib - 6]
    if 26 <= ib <= 31:
        j = 20 + 2 * (ib - 26)
        return [j, j + 1]
    return []


def _emit_body(nc, tc, stack, C, xt_d, gu_d, dt_d, y_d):
    tiles = _ctiles(C)
    NT = len(tiles)
    silu = mybir.ActivationFunctionType.Silu

    with (
        tc.tile_pool(name="wpool", bufs=8) as wpool,
        tc.tile_pool(name="xpool", bufs=2) as xpool,
        tc.tile_pool(name="dpool", bufs=1) as dpool,
        tc.tile_pool(name="hpool", bufs=2) as hpool,
        tc.tile_pool(name="spool", bufs=3) as spool,
        tc.tile_pool(name="ypool", bufs=3) as ypool,
        tc.tile_pool(name="psum", bufs=2, space="PSUM") as psum,
    ):
        gus = {}

        def gu_load(t, ib):
            g = wpool.tile([128, 2, KCH, 128], BF16, tag="gu")
            nc.sync.dma_start(g[:], gu_d[ib])
            gus[(t, ib)] = g

        PF = 6  # gate/up prefetch depth (blocks in flight ahead of the PE)
        xts = [None] * NT
        xts[0] = xpool.tile([128, KCH, CT], BF16, tag="xt", name="xt0")
        nc.scalar.dma_start(xts[0][:], xt_d[0])
        for j in range(PF):
            gu_load(0, j)

        # D^T resident for the whole rep: [128, IB, H] bf16, loaded in
        # 32 chunks interleaved through tile 0's stage 1 so the transfers
        # never starve the gate/up weight stream.
        dtt = dpool.tile([128, IB, H], BF16, tag="dt")

        for t, (c0, cs) in enumerate(tiles):
            xt = xts[t]
            hhs = []
            for ib in range(IB):
                gu = gus.pop((t, ib))
                if ib + PF < IB:
                    gu_load(t, ib + PF)
                a1 = psum.tile([128, CT], F32, tag="a1", bufs=3)
                for k in range(KCH):
                    nc.tensor.matmul(
                        a1[:, :cs], gu[:, 0, k, :], xt[:, k, :cs],
                        start=(k == 0), stop=(k == KCH - 1),
                    )
                a2 = psum.tile([128, CT], F32, tag="a2", bufs=3)
                for k in range(KCH):
                    nc.tensor.matmul(
                        a2[:, :cs], gu[:, 1, k, :], xt[:, k, :cs],
                        start=(k == 0), stop=(k == KCH - 1),
                    )
                sl = spool.tile([128, CT], F32, tag="silu")
                nc.scalar.activation(sl[:, :cs], a1[:, :cs], silu)
                if ib == 20 and t + 1 < NT:
                    # prefetch next tile's tokens mid-stage-1; the
                    # Activation queue has transfer slack here and the
                    # coarse tile-level dep gates stage 1 of t+1 on this
                    xts[t + 1] = xpool.tile([128, KCH, CT], BF16, tag="xt", name="xtn")
                    nc.scalar.dma_start(xts[t + 1][:], xt_d[t + 1])
                if t == 0 and ib >= 6:
                    # D^T chunks ride the Activation HWDGE queue (the SP
                    # queue is saturated by the gate/up stream; SWDGE via
                    # gpsimd measured far slower than its cost model).
                    # Start at ib>=6 so the cross-rep WAR wait (previous
                    # rep's stage-2 reads of dtt) is already satisfied and
                    # never blocks the silu stream queued behind it.
                    for j in _dt_chunks_at(ib):
                        nc.scalar.dma_start(dtt[:, j, :], dt_d[:, j, :])
                hh = hpool.tile([128, CT], BF16, tag=f"hh{ib}")
                nc.vector.tensor_mul(hh[:, :cs], sl[:, :cs], a2[:, :cs])
                hhs.append(hh)

            # prefetch next tile's inputs before stage 2 keeps PE fed
            # across the tile boundary
            if t + 1 < NT:
                for j in range(PF):
                    gu_load(t + 1, j)

            # stage 2: Y[c, h] = Hh @ D^T  (contract I)
            for cs0 in range(0, cs, 128):
                yt = ypool.tile([128, H], F32, tag="yt")
                for hi, h0 in enumerate(range(0, H, 512)):
                    py = psum.tile([128, 512], F32, tag="py")
                    for ic in range(IB):
                        nc.tensor.matmul(
                            py[:],
                            hhs[ic][:, cs0 : cs0 + 128],
                            dtt[:, ic, h0 : h0 + 512],
                            start=(ic == 0), stop=(ic == IB - 1),
                        )
                    nc.scalar.copy(yt[:, h0 : h0 + 512], py[:])
                    nc.scalar.dma_start(
                        y_d[c0 + cs0 : c0 + cs0 + 128, h0 : h0 + 512],
                        yt[:, h0 : h0 + 512],
                    )


def _routing(x, router_w):
    """Replicate the reference's routing decisions with identical jax ops."""
    import jax
    import jax.numpy as jnp

    xf = jnp.asarray(x).reshape(-1, H)
    logits = xf @ jnp.asarray(router_w).T
    probs = jax.nn.softmax(logits, axis=-1)
    topk_p, topk_i = jax.lax.top_k(probs, TOPK)
    topk_p = topk_p / topk_p.sum(axis=-1, keepdims=True)
    return np.asarray(topk_p), np.asarray(topk_i)


def prepare(x, router_w, gate_w, up_w, down_w):
    """Host-side dispatch: returns (nc, in_maps, combine) where combine maps
    the per-core device outputs to the full [B,S,H] result."""
    topk_p, topk_i = _routing(x, router_w)
    xf = np.ascontiguousarray(np.asarray(x, dtype=np.float32).reshape(T, H))

    idxs, weights = [], []
    for e in range(E):
        sel = topk_i == e
        mask = sel.any(axis=-1)
        w_tok = (topk_p * sel).sum(axis=-1)
        cnt = int(mask.sum())
        mean_w = float(w_tok.sum() / max(cnt, 1)) if cnt > 0 else 0.0
        idxs.append(np.nonzero(mask)[0])
        weights.append(np.float32(mean_w))

    cmax = max(len(ix) for ix in idxs)
    C = ((cmax + 127) // 128) * 128
    NT = len(_ctiles(C))

    xf_bf = xf.astype(ml_dtypes.bfloat16)
    in_maps = []
    for e in range(E):
        ix = idxs[e]
        # X^T per tile: [NT, 128(h-sub), KCH(k), CT(c)], h = 128k + p.
        # Tile t covers tokens [c0, c0+CT) of the padded stream (tiles can
        # be shorter than CT; the kernel only reads [:cs] of each).
        xpad = np.zeros((C + CT, H), dtype=ml_dtypes.bfloat16)
        xpad[: len(ix)] = xf_bf[ix]
        xt = np.ascontiguousarray(
            np.stack(
                [
                    xpad[c0 : c0 + CT].reshape(CT, KCH, 128).transpose(2, 1, 0)
                    for c0, cs in _ctiles(C)
                ]
            )
        )
        # G^T/U^T packed together: [IB, 128(p), 2, KCH(k), 128(i)], h = 128k+p
        gT = np.asarray(gate_w[e], dtype=np.float32).T.astype(ml_dtypes.bfloat16)
        uT = np.asarray(up_w[e], dtype=np.float32).T.astype(ml_dtypes.bfloat16)
        gt = gT.reshape(KCH, 128, IB, 128).transpose(2, 1, 0, 3)
        ut = uT.reshape(KCH, 128, IB, 128).transpose(2, 1, 0, 3)
        gu = np.ascontiguousarray(np.stack([gt, ut], axis=2))
        # D^T: [128(p over I-sub), IB, H] with i = 128*ic + p
        dT = np.asarray(down_w[e], dtype=np.float32).T.astype(ml_dtypes.bfloat16)
        dt = np.ascontiguousarray(dT.reshape(IB, 128, H).transpose(1, 0, 2))
        in_maps.append({"xt": xt, "gu": gu, "dt": dt})

    nc = build_program(C)

    def combine(results):
        out = np.zeros((T, H), dtype=np.float32)
        for e in range(E):
            ix = idxs[e]
            y = results[e]["y"]
            out[ix] += weights[e] * y[: len(ix)]
        return out.reshape(B, S, H)

    return nc, in_maps, combine


def kernel(x, router_w, gate_w, up_w, down_w):
    nc, in_maps, combine = prepare(x, router_w, gate_w, up_w, down_w)
    res = run_bass_kernel_spmd(nc, in_maps, list(range(8)))
    return combine(res.results)

